# revision 1
# baseline (speedup 1.0000x reference)
"""NSVQ (noise-substitution VQ) Trainium2 kernel.

Problem: out = decode(x + ||x - c_nearest|| / (||r||+eps) * r), where
x = encode(input). Key identity used: ||x - c_nearest||^2 =
||x||^2 - 2*max_k(x.c_k - 0.5||c_k||^2), so no argmin / gather is needed.

Sharding: data-parallel over tokens. Core i handles batches [2i, 2i+1]
(4096 tokens each); codebook + projection weights replicated.

Layout per core (tokens chunked 128 at a time, tiled 512 at a time):
  encode:  x_T[64, 512] = W_in^T @ inp (PSUM, float32r matmuls)
  xhat:    SBUF copy of x_T + b_in, with a ones row (row 64) so the
           distance matmul folds -0.5||c||^2 via an augmented codebook.
  dist:    scores [128 tok, 1024 codes] in one 2-bank PSUM tile (two
           float32r matmuls vs the augmented codebook), then a single
           DVE reduce_max -> smax per chunk. (tensor_tensor_reduce and
           Pool-engine free-axis reduce are broken on this HW/toolchain.)
  norms:   ||x||^2: 4 PE transposes/tile -> one ACT Square -> one DVE
           3D reduce_sum; ||r||^2: DVE STT r*r with sum-accumulate.
  scale:   sqrt(relu(nsq - 2 smax) * recip(rsq)), batched per tile.
  decode:  scaled r (GPSIMD mult, stride-0 broadcast of scale) is
           transposed on PE accumulating onto x_T PSUM -> q_T;
           out = W_out_aug^T @ [q_T; ones] (b_out folded).
"""

import numpy as np
from contextlib import ExitStack

B, DIM, T = 16, 256, 2048
K, D = 1024, 64
NCORES = 8
BPC = B // NCORES          # batches per core
NTOK = BPC * T             # tokens per core
TTILE = 512                # tokens per tile
NTILES = NTOK // TTILE     # 8
CHUNK = 128
CPT = TTILE // CHUNK       # chunks per tile = 4
NCHUNK = NTOK // CHUNK     # 32
BATCH_TILES = 1            # tiles per scale batch
EPS = 1e-12

_CACHE = {}

import os
ABLATE = set(os.environ.get("KABLATE", "").split(",")) - {""}


def _emit(ctx, tc, aps):
    import concourse.bass as bass
    from concourse import mybir

    nc = tc.nc
    f32 = mybir.dt.float32
    f32r = mybir.dt.float32r
    AX = mybir.AluOpType
    AF = mybir.ActivationFunctionType
    ts = bass.ts

    inp, rr, win, binc, cba, woa, eye, out = (
        aps["inp"], aps["rr"], aps["win"], aps["binc"], aps["cba"],
        aps["woa"], aps["eye"], aps["out"],
    )

    # ---- pools ----
    const = ctx.enter_context(tc.tile_pool(name="const", bufs=1))
    persist = ctx.enter_context(tc.tile_pool(name="persist", bufs=1))
    inpool = ctx.enter_context(tc.tile_pool(name="inpool", bufs=4))
    d1pool = ctx.enter_context(tc.tile_pool(name="d1pool", bufs=5))
    scrpool = ctx.enter_context(tc.tile_pool(name="scrpool", bufs=2))
    sqpool = ctx.enter_context(tc.tile_pool(name="sqpool", bufs=4))
    srpool = ctx.enter_context(tc.tile_pool(name="srpool", bufs=3))
    opool = ctx.enter_context(tc.tile_pool(name="opool", bufs=3))

    xpsum = ctx.enter_context(tc.tile_pool(name="xpsum", bufs=2, space="PSUM"))
    dpsum = ctx.enter_context(tc.tile_pool(name="dpsum", bufs=2, space="PSUM"))
    tpsum = ctx.enter_context(tc.tile_pool(name="tpsum", bufs=1, space="PSUM"))
    opsum = ctx.enter_context(tc.tile_pool(name="opsum", bufs=1, space="PSUM"))

    # ---- constants ----
    w0 = const.tile([128, D], f32r, tag="w0")
    nc.sync.dma_start(w0[:], win[0:128, :])
    w1 = const.tile([128, D], f32r, tag="w1")
    nc.sync.dma_start(w1[:], win[128:256, :])
    binc_sb = const.tile([D, 1], f32, tag="binc")
    nc.sync.dma_start(binc_sb[:], binc[:])
    cba_sb = const.tile([D + 1, K], f32r, tag="cba")
    nc.sync.dma_start(cba_sb[:], cba[:])
    woa_sb = const.tile([D + 1, DIM], f32r, tag="woa")
    nc.sync.dma_start(woa_sb[:], woa[:])
    eye_sb = const.tile([128, 128], f32r, tag="eye")
    nc.sync.dma_start(eye_sb[:], eye[:])

    # whole random-vector slice, token-major [128, NCHUNK, 64]
    rall = persist.tile([128, NCHUNK, D], f32, tag="rall")
    nc.sync.dma_start(rall[:], rr[:])

    # persistent x-hat / q-hat tiles with a ones row at row 64
    xh = [persist.tile([D + 1, TTILE], f32r, tag=f"xh{n}", name=f"xh{n}")
          for n in range(4)]
    qh = [persist.tile([D + 1, TTILE], f32r, tag=f"qh{n}", name=f"qh{n}")
          for n in range(4)]
    for t_ in xh + qh:
        nc.gpsimd.memset(t_[D:D + 1, :].bitcast(f32), 1.0)

    zeros32 = persist.tile([128, NCHUNK], f32, tag="zeros32", name="zeros32")
    nc.gpsimd.memset(zeros32[:], 0.0)

    # per-token stats, chunk j lives in column j
    stats = {}
    for nm in ("smax", "nsq", "rsq", "resid2", "nres", "nrand", "recd",
               "scalev"):
        stats[nm] = persist.tile([128, NCHUNK], f32, tag=nm, name=nm)
    smax, nsq, rsq = stats["smax"], stats["nsq"], stats["rsq"]
    resid2, nres, nrand = stats["resid2"], stats["nres"], stats["nrand"]
    recd, scalev = stats["recd"], stats["scalev"]

    Xtiles = {}

    def phase_a(i):
        b, t4 = divmod(i, NTILES // BPC)
        t0 = t4 * TTILE
        xt = xh[i % 4]

        in0 = inpool.tile([128, TTILE], f32r, tag="in0")
        nc.sync.dma_start(in0[:], inp[b, 0:128, t0:t0 + TTILE])
        in1 = inpool.tile([128, TTILE], f32r, tag="in1")
        nc.sync.dma_start(in1[:], inp[b, 128:256, t0:t0 + TTILE])

        X = xpsum.tile([D, TTILE], f32, tag="X")
        Xtiles[i] = X
        nc.tensor.matmul(X[:], w0[:], in0[:],
                         start=True, stop=False)
        nc.tensor.matmul(X[:], w1[:], in1[:],
                         start=False, stop=True)
        # evacuate + add b_in; row 64 of xt stays = ones
        nc.scalar.activation(xt[0:D, :], X[:], AF.Identity, bias=binc_sb[:])

        for j4 in range(CPT):
            j = CPT * i + j4
            xsl = xt[0:D + 1, ts(j4, CHUNK)]
            # pair-folded scores: E = A + |Dm| = max(s_2k, s_2k+1), then
            # one DVE max-reduce over 512 pairs
            dd = dpsum.tile([128, K], f32, tag="d")
            nc.tensor.matmul(dd[:, 0:K // 2], xsl, cba_sb[:, 0:K // 2],
                             start=True, stop=True)
            nc.tensor.matmul(dd[:, K // 2:K], xsl, cba_sb[:, K // 2:K],
                             start=True, stop=True)
            nc.vector.reduce_max(smax[:, j:j + 1], dd[:],
                                 axis=mybir.AxisListType.X)



        # ||r||^2 batched per tile: GP squares -> one DVE 3D reduce-sum
        if "rsq" not in ABLATE:
            rsqsq = sqpool.tile([128, CPT * D], f32, tag="rsqsq")
            rsl = rall[:, ts(i, CPT), :]
            nc.gpsimd.tensor_tensor(
                rsqsq[:].rearrange("p (c d) -> p c d", c=CPT), rsl, rsl,
                op=AX.mult)
            nc.vector.reduce_sum(
                rsq[:, ts(i, CPT)],
                rsqsq[:].rearrange("p (c d) -> p c d", c=CPT),
                axis=mybir.AxisListType.X)

        # ||x||^2: 4 PE transposes -> per-chunk ACT Square with accumulate
        if "nsq" not in ABLATE:
            XT4 = tpsum.tile([128, CPT * D], f32, tag="xtm")
            for j4 in range(CPT):
                nc.tensor.transpose(XT4[:, ts(j4, D)].bitcast(f32r),
                                    xt[0:D, ts(j4, CHUNK)], eye_sb[0:D, 0:D])
            sq4 = sqpool.tile([128, CPT * D], f32, tag="sq4")
            for j4 in range(CPT):
                nc.scalar.activation(sq4[:, ts(j4, D)], XT4[:, ts(j4, D)],
                                     AF.Square,
                                     accum_out=nsq[:, CPT * i + j4:CPT * i + j4 + 1])

    def scale_math(k):
        if "smath" in ABLATE:
            return
        c8 = ts(k, BATCH_TILES * CPT)
        # recip(rsq) does not depend on smax -> off the critical path
        nc.vector.reciprocal(recd[:, c8], rsq[:, c8])
        # resid^2 = nsq - 2*smax, clamped at 0
        nc.vector.scalar_tensor_tensor(resid2[:, c8], smax[:, c8], -2.0,
                                       nsq[:, c8], AX.mult, AX.add)
        nc.vector.tensor_scalar_max(resid2[:, c8], resid2[:, c8], 0.0)
        nc.gpsimd.tensor_tensor(nres[:, c8], resid2[:, c8], recd[:, c8],
                                op=AX.mult)
        nc.scalar.sqrt(scalev[:, c8], nres[:, c8])

    def phase_b(i):
        b, t4 = divmod(i, NTILES // BPC)
        t0 = t4 * TTILE
        X = Xtiles.pop(i)
        qt = qh[i % 4]

        # scaled r, one TT op per tile: broadcast scale along d via stride-0
        srt = srpool.tile([128, CPT, D], f32r, tag="srt")
        if "srt" not in ABLATE:
            scl = scalev[:, ts(i, CPT)].unsqueeze(2).broadcast_to([128, CPT, D])
            nc.gpsimd.tensor_tensor(srt[:], rall[:, ts(i, CPT), :], scl,
                                    op=AX.mult)
        # transpose-accumulate onto X: q_T = x_T + (scale*r)^T
        if "taccum" not in ABLATE:
            for j4 in range(CPT):
                nc.tensor.matmul(X[:, ts(j4, CHUNK)].bitcast(f32r),
                                 srt[:, j4, :], eye_sb[:], is_transpose=True,
                                 start=False, stop=(j4 == CPT - 1),
                                 skip_group_check=True)
        nc.scalar.activation(qt[0:D, :], X[:], AF.Identity, bias=binc_sb[:])

        osb = opool.tile([128, 2 * TTILE], f32, tag="osb")
        for m in range(2):
            O = opsum.tile([128, TTILE], f32, tag="O", name="O")
            nc.tensor.matmul(O[:], woa_sb[:, ts(m, 128)], qt[:],
                             start=True, stop=True)
            nc.scalar.copy(osb[:, ts(m, TTILE)], O[:])
        nc.sync.dma_start(out[b, 0:128, t0:t0 + TTILE], osb[:, 0:TTILE])
        nc.sync.dma_start(out[b, 128:256, t0:t0 + TTILE], osb[:, TTILE:2 * TTILE])

    for _rep in range(int(os.environ.get("KREPEAT", "1"))):
        for k in range(NTILES // BATCH_TILES):
            for i in range(k * BATCH_TILES, (k + 1) * BATCH_TILES):
                phase_a(i)
            scale_math(k)
            for i in range(k * BATCH_TILES, (k + 1) * BATCH_TILES):
                phase_b(i)


def build():
    if "nc" in _CACHE:
        return _CACHE["nc"]
    from concourse import bacc, mybir
    import concourse.tile as tile

    nc = bacc.Bacc("TRN2", target_bir_lowering=False, debug=False,
                   enable_asserts=False, num_devices=NCORES)
    f32 = mybir.dt.float32
    f32r = mybir.dt.float32r
    aps = {
        "inp": nc.dram_tensor("inp", [BPC, DIM, T], f32r,
                              kind="ExternalInput").ap(),
        "rr": nc.dram_tensor("rr", [128, NCHUNK, D], f32,
                             kind="ExternalInput").ap(),
        "win": nc.dram_tensor("win", [DIM, D], f32r, kind="ExternalInput").ap(),
        "binc": nc.dram_tensor("binc", [D, 1], f32, kind="ExternalInput").ap(),
        "cba": nc.dram_tensor("cba", [D + 1, K], f32r,
                              kind="ExternalInput").ap(),
        "woa": nc.dram_tensor("woa", [D + 1, DIM], f32r,
                              kind="ExternalInput").ap(),
        "eye": nc.dram_tensor("eye", [128, 128], f32r,
                              kind="ExternalInput").ap(),
        "out": nc.dram_tensor("out", [BPC, DIM, T], f32,
                              kind="ExternalOutput").ap(),
    }
    with tile.TileContext(nc) as tc:
        with ExitStack() as ctx:
            _emit(ctx, tc, aps)
    nc.compile()
    _CACHE["nc"] = nc
    return nc


def make_in_maps(input_data, codebooks, W_in, b_in, W_out, b_out,
                 random_vector):
    f = np.float32
    cb = np.asarray(codebooks, f)
    cba = np.concatenate([cb.T, (-0.5 * (cb * cb).sum(1))[None, :]],
                         0).astype(f)  # [65, K] augmented codebook
    woa = np.concatenate([np.asarray(W_out, f),
                          np.asarray(b_out, f)[None, :]], 0).astype(f)
    eye = np.eye(128, dtype=f)
    binc = np.ascontiguousarray(np.asarray(b_in, f).reshape(D, 1))
    win = np.ascontiguousarray(np.asarray(W_in, f))
    rv = np.asarray(random_vector, f).reshape(NCORES, NCHUNK, 128, D)
    in_maps = []
    for i in range(NCORES):
        rr = np.ascontiguousarray(rv[i].transpose(1, 0, 2))  # [128, NCHUNK, D]
        in_maps.append({
            "inp": np.ascontiguousarray(input_data[BPC * i:BPC * (i + 1)],
                                        dtype=f),
            "rr": rr,
            "win": win, "binc": binc, "cba": cba, "woa": woa, "eye": eye,
        })
    return in_maps


def kernel(input_data, codebooks, W_in, b_in, W_out, b_out, random_vector,
           **kwargs):
    from concourse.bass_utils import run_bass_kernel_spmd

    nc = build()
    in_maps = make_in_maps(input_data, codebooks, W_in, b_in, W_out, b_out,
                           random_vector)
    res = run_bass_kernel_spmd(nc, in_maps, core_ids=list(range(NCORES)),
                               **kwargs)
    out = np.concatenate([res.results[i]["out"] for i in range(NCORES)],
                         axis=0)
    _CACHE["last_res"] = res
    return out


if __name__ == "__main__":
    nc = build()
    print("compiled OK")

